# revision 10
# baseline (speedup 1.0000x reference)
"""CASSI layer kernel for Trainium2 (8 NeuronCores, Bass/Tile) — v2.

Math (matches the reference nn_CASSI_layer):
    H2[m,n,s]        = H[0,m,n,0,s]
    Y[b,m,n+l,s]    += H2[m,n,s] * x[b,m,n,l]            (shear-sum, l in [0,24))
    sigm             = sum(Y^2) / (M*W*B*10^(40/10))
    Yn               = Y + sqrt(sigm) * noise_eps         (noise_eps broadcast over s)
    X[b,m,n,l]       = sum_s H2[m,n,s] * Yn[b,m,n+l,s]
    out              = X / max(X)

Distribution: 4*256 = 1024 independent (b, m) rows; core c takes 128 rows
(b = c//2, m in [128*(c%2), ...+128)) mapped to the 128 SBUF partitions.

v2 design (vs the v1 all-DVE/GPSIMD kernel at 250us):
  * Transposed free-dim layouts put the shared n axis LAST and PACKED, so
    every VectorE multiply keeps the fp16 2x perf mode while the per-s /
    per-l broadcast rides a stride-0 MIDDLE dim — the ScalarE broadcast
    copies of v1 disappear entirely.
  * ALL accumulation moves to the idle TensorEngine as identity-weight
    matmuls accumulating in PSUM (fp32):
      stage 2: for each shot s, the 24 shear windows Y_s[l:l+256] += P_s[l]
      are 24 accumulating matmuls into a one-bank [p, 279] PSUM tile (row 0
      is W-wide via a zeroed gutter so start=True covers the full bank);
      stage 4: for each band l, X_l[p, 256] accumulates the 22 s-slices.
  * ScalarE only evacuates PSUM->SBUF (fp32->fp16) and does the Square
    accumulation for sigma.
  * GPSIMD (tensor_tensor, 0.42 eff) takes a tuned subset of the multiplies;
    its products are issued a few slots early so the in-order PE group
    stream never stalls on them.
  * H is pre-scaled by 256 on the host (X/max(X) is scale-invariant) so all
    fp16 intermediates sit in a healthy range.
  * PE p-state: dummy identity matmuls during the initial DMA window and the
    stage-2/4 boundary keep the tensor clock ramped.

The two global scalar couplings (sigm, max) are linearized out of the device
kernel: X = X0 + sqrt(sigm)*Xn with X0 the noise-free result (device) and
Xn = hsum*noise window (host outer product); the host applies sigma and the
global max normalization exactly as v1 did.
"""

from contextlib import ExitStack

import numpy as np

import concourse.bass as bass
import concourse.bacc as bacc
import concourse.tile as tile
from concourse import mybir
from concourse.bass_utils import run_bass_kernel_spmd

B, M, L, S = 4, 256, 24, 22
W = M + L - 1  # 279
N_CORES = 8
ROWS = 128
NOISE_DB = 40.0
H_SCALE = 256.0  # host pre-scale on H; output is scale-invariant

_F32 = mybir.dt.float32
_F16 = mybir.dt.float16


def build_bass(
    gps2=(5, 10, 15, 20),         # stage-2 s values multiplied on GPSIMD
    gps4=(4, 8, 12, 16, 20),      # stage-4 l values multiplied on GPSIMD
    look2=4,                      # how many slots early GPSIMD s-muls issue
    look4=4,
    p_bufs=3, q_bufs=3,           # DVE product pools
    gp_bufs=2, gq_bufs=2,         # GPSIMD product pools
    ypsum_bufs=4, xpsum_bufs=3,
    dummies_start=40, dummies_mid=16,
    h_chunk=4,                    # first h rows DMA'd separately
    x0_chunks=3,
) -> bass.Bass:
    nc = bacc.Bacc()
    x_in = nc.declare_dram_parameter("x_in", [ROWS, L, M], _F16, isOutput=False)
    h_in = nc.declare_dram_parameter("h_in", [ROWS, S, M], _F16, isOutput=False)
    id_in = nc.declare_dram_parameter("id_in", [ROWS, ROWS], _F16, isOutput=False)
    x0_out = nc.declare_dram_parameter("x0_out", [ROWS, L, M], _F16, isOutput=True)
    ss_out = nc.declare_dram_parameter("ss_out", [ROWS, 1], _F32, isOutput=True)

    mult = mybir.AluOpType.mult

    with tile.TileContext(nc) as tc, ExitStack() as ctx:
        main = ctx.enter_context(tc.tile_pool(name="main", bufs=1))
        pp = ctx.enter_context(tc.tile_pool(name="pp", bufs=p_bufs))
        qp = ctx.enter_context(tc.tile_pool(name="qp", bufs=q_bufs))
        gpp = ctx.enter_context(tc.tile_pool(name="gpp", bufs=gp_bufs))
        gqp = ctx.enter_context(tc.tile_pool(name="gqp", bufs=gq_bufs))
        psum_d = ctx.enter_context(tc.tile_pool(name="psum_d", bufs=1, space="PSUM"))
        psum_y = ctx.enter_context(
            tc.tile_pool(name="psum_y", bufs=ypsum_bufs, space="PSUM")
        )
        psum_x = ctx.enter_context(
            tc.tile_pool(name="psum_x", bufs=xpsum_bufs, space="PSUM")
        )

        xs = main.tile([ROWS, L, M], _F16, tag="xs")
        hs = main.tile([ROWS, S, M], _F16, tag="hs")
        ident = main.tile([ROWS, ROWS], _F16, tag="ident")
        ys = main.tile([ROWS, S, W], _F16, tag="ys")
        x0 = main.tile([ROWS, L, M], _F16, tag="x0")
        ss = main.tile([ROWS, 1], _F32, tag="ss")

        # input DMAs: ident first (PE dummies), then x, then h (first rows
        # separately so stage-2 s=0 can begin before the whole of h lands)
        nc.sync.dma_start(out=ident, in_=id_in[:])
        nc.sync.dma_start(out=xs, in_=x_in[:])
        nc.sync.dma_start(out=hs[:, 0:h_chunk, :], in_=h_in[:, 0:h_chunk, :])
        nc.sync.dma_start(out=hs[:, h_chunk:, :], in_=h_in[:, h_chunk:, :])

        # PE warm-up: keep the tensor engine busy while DMAs land so the
        # p-state ramp is done when real groups arrive.
        scratch = psum_d.tile([ROWS, 512], _F32, tag="scratch")
        for _ in range(dummies_start):
            nc.tensor.matmul(
                out=scratch[:, 0:ROWS], lhsT=ident, rhs=ident, start=True, stop=True
            )

        # zero the gutters of every product-pool buffer once (cheap, GPSIMD
        # is idle during the DMA window).  P tiles are [p, L, W]: each l-row
        # is W wide; the mul writes [0:M), gutter [M:W) stays zero so the
        # j=0 matmul can cover the full W window with start=True.
        pts, gpts = [], []
        for i in range(p_bufs):
            pt = pp.tile([ROWS, L, W], _F16, tag="pt")
            nc.gpsimd.memset(pt[:, :, M:], 0.0)
            pts.append(pt)
        for i in range(gp_bufs):
            gt = gpp.tile([ROWS, L, W], _F16, tag="gt")
            nc.gpsimd.memset(gt[:, :, M:], 0.0)
            gpts.append(gt)

        def h_row_bcast(s: int) -> bass.AP:
            # hs[:, s, :] broadcast along a leading l axis: [ROWS, L, M]
            return bass.AP(
                tensor=hs.tensor,
                offset=hs.offset + s * M,
                ap=[hs.ap[0], [0, L], [1, M]],
            )

        def ys_window(l: int) -> bass.AP:
            # ys[:, :, l:l+M]: [ROWS, S, M]
            return bass.AP(
                tensor=ys.tensor,
                offset=ys.offset + l,
                ap=[ys.ap[0], [W, S], [1, M]],
            )

        # ---------------- stage 2: Y[s, n+l] += x[l, n] * H[s, n] ----------
        GPS2 = set(gps2)
        dve_i = 0  # rotates pts
        gps_i = 0
        emitted_gps_mul: dict[int, bass.AP] = {}

        def emit_gps2_mul(s: int):
            nonlocal gps_i
            gt = gpts[gps_i % gp_bufs]
            gps_i += 1
            nc.gpsimd.tensor_tensor(
                out=gt[:, :, 0:M], in0=xs, in1=h_row_bcast(s), op=mult
            )
            emitted_gps_mul[s] = gt

        order = list(range(S))
        for slot, s in enumerate(order):
            if s in GPS2:
                if s not in emitted_gps_mul:
                    emit_gps2_mul(s)
                pt = emitted_gps_mul[s]
            else:
                pt = pts[dve_i % p_bufs]
                dve_i += 1
                nc.vector.tensor_tensor(
                    out=pt[:, :, 0:M], in0=xs, in1=h_row_bcast(s), op=mult
                )
            ypsum = psum_y.tile([ROWS, W], _F32, tag="ypsum")
            nc.tensor.matmul(
                out=ypsum, lhsT=ident, rhs=pt[:, 0, 0:W], start=True, stop=False
            )
            for j in range(1, L):
                nc.tensor.matmul(
                    out=ypsum[:, j : j + M],
                    lhsT=ident,
                    rhs=pt[:, j, 0:M],
                    start=False,
                    stop=(j == L - 1),
                )
            nc.scalar.copy(out=ys[:, s, :], in_=ypsum)
            # issue upcoming GPSIMD muls AFTER this slot's PE group so the
            # gpts ring is never clobbered ahead of its reader
            la = slot + look2
            if la < S and order[la] in GPS2 and order[la] not in emitted_gps_mul:
                emit_gps2_mul(order[la])

        # keep PE warm across the phase boundary
        for _ in range(dummies_mid):
            nc.tensor.matmul(
                out=scratch[:, 0:ROWS], lhsT=ident, rhs=ident, start=True, stop=True
            )

        # ---------------- stage 4: X[l, n] = sum_s H[s, n] * Y[s, n+l] ------
        GPS4 = set(gps4)
        emitted_gps4: dict[int, bass.AP] = {}
        qts = [qp.tile([ROWS, S, M], _F16, tag="qt", name=f"qt{i}") for i in range(q_bufs)]
        gqts = [gqp.tile([ROWS, S, M], _F16, tag="gqt", name=f"gqt{i}") for i in range(gq_bufs)]
        dve4_i = 0
        gps4_i = 0

        def emit_gps4_mul(l: int):
            nonlocal gps4_i
            gq = gqts[gps4_i % gq_bufs]
            gps4_i += 1
            nc.gpsimd.tensor_tensor(out=gq, in0=hs, in1=ys_window(l), op=mult)
            emitted_gps4[l] = gq

        order4 = list(range(L))
        # sigma scratch: Square writes Y^2 here; accum_out gives sum per row
        sq = main.tile([ROWS, S, W], _F16, tag="sq")
        sq_after = 2  # emit the Square after this many stage-4 evacs
        chunk_ends = {(c + 1) * L // x0_chunks - 1: c for c in range(x0_chunks)}
        for slot, l in enumerate(order4):
            if l in GPS4:
                if l not in emitted_gps4:
                    emit_gps4_mul(l)
                qt = emitted_gps4[l]
            else:
                qt = qts[dve4_i % q_bufs]
                dve4_i += 1
                nc.vector.tensor_tensor(out=qt, in0=hs, in1=ys_window(l), op=mult)
            xpsum = psum_x.tile([ROWS, M], _F32, tag="xpsum")
            for j in range(S):
                nc.tensor.matmul(
                    out=xpsum,
                    lhsT=ident,
                    rhs=qt[:, j, :],
                    start=(j == 0),
                    stop=(j == S - 1),
                )
            nc.scalar.copy(out=x0[:, l, :], in_=xpsum)
            la = slot + look4
            if la < L and order4[la] in GPS4 and order4[la] not in emitted_gps4:
                emit_gps4_mul(order4[la])
            if slot == sq_after:
                nc.scalar.activation(
                    out=sq,
                    in_=ys,
                    func=mybir.ActivationFunctionType.Square,
                    accum_out=ss,
                )
                nc.sync.dma_start(out=ss_out[:], in_=ss)
            if slot in chunk_ends:
                c = chunk_ends[slot]
                lo = c * L // x0_chunks
                nc.sync.dma_start(
                    out=x0_out[:, lo : slot + 1, :], in_=x0[:, lo : slot + 1, :]
                )

    nc.finalize()
    return nc


def shard_inputs(x: np.ndarray, H: np.ndarray) -> list[dict[str, np.ndarray]]:
    H2 = (H[0, :, :, 0, :] * H_SCALE).astype(np.float16)  # (M, M, S)
    x16 = x.astype(np.float16)
    ident = np.eye(ROWS, dtype=np.float16)
    in_maps = []
    for c in range(N_CORES):
        b, half = c // 2, c % 2
        m0 = half * ROWS
        # xsT[p, l, n] = x[b, m0+p, n, l]
        xT = np.ascontiguousarray(x16[b, m0 : m0 + ROWS].transpose(0, 2, 1))
        # hsT[p, s, n] = H2[m0+p, n, s]
        hT = np.ascontiguousarray(H2[m0 : m0 + ROWS].transpose(0, 2, 1))
        in_maps.append({"x_in": xT, "h_in": hT, "id_in": ident})
    return in_maps


def finalize(
    results: list[dict[str, np.ndarray]],
    H: np.ndarray,
    noise_eps: np.ndarray,
) -> np.ndarray:
    X0 = np.empty((B, M, M, L), np.float32)
    sumsq = 0.0
    for c in range(N_CORES):
        b, half = c // 2, c % 2
        m0 = half * ROWS
        # x0T[p, l, n] -> X0[b, m, n, l]
        X0[b, m0 : m0 + ROWS] = (
            results[c]["x0_out"].astype(np.float32).transpose(0, 2, 1)
        )
        sumsq += results[c]["ss_out"].sum(dtype=np.float64)
    # sumsq is for scaled H (Y' = 256 Y); keep everything in scaled units —
    # the final X/max(X) is invariant to the global 256^2 factor.
    sigm = sumsq / (M * W * B * 10.0 ** (NOISE_DB / 10.0))

    H2s = H[0, :, :, 0, :].astype(np.float32) * np.float32(H_SCALE)
    hsum = H2s.sum(axis=-1)  # (M, M), scaled
    nwin = np.lib.stride_tricks.sliding_window_view(
        noise_eps[:, :, :, 0].astype(np.float32), L, axis=2
    )
    X = X0 + np.float32(np.sqrt(sigm)) * (hsum[None, :, :, None] * nwin)
    X = X.astype(np.float32, copy=False)
    return X / X.max()


_NC_CACHE: bass.Bass | None = None


def kernel(x: np.ndarray, H: np.ndarray, noise_eps: np.ndarray) -> np.ndarray:
    global _NC_CACHE
    x = np.asarray(x, dtype=np.float32)
    H = np.asarray(H, dtype=np.float32)
    noise_eps = np.asarray(noise_eps, dtype=np.float32)
    if _NC_CACHE is None:
        _NC_CACHE = build_bass()
    in_maps = shard_inputs(x, H)
    res = run_bass_kernel_spmd(_NC_CACHE, in_maps, core_ids=list(range(N_CORES)))
    return finalize(res.results, H, noise_eps)
